# revision 58
# baseline (speedup 1.0000x reference)
"""Trainium2 Bass kernel for nn_CPPN: 3-layer MLP (4->64->64->3, tanh) over
1M pixels + global min/max normalization, data-parallel over 8 NeuronCores.

Layout strategy (per core, NPIX = 131072 pixels):
  - pixels split into 16 subsets of 8192; subset s (g = s%4, q = s//4) lives
    at partitions 32g+8q+{0..7} of xin (bf16 hi at +0..3, lo at +4..7).
  - layer-1: per tile t = (g, c) TWO matmuls (one per 512-col half v),
    lhsT [32,128] covering subsets q = 2v, 2v+1 -> p1 [128, 1024] with
    features on partitions (row-half a = subset q = 2v+a).
  - layer-2: ONE blockdiag [128,128] matmul per 512-col half (both row
    halves at once) -> 2 matmuls/tile instead of 4.
  - layer-3: block-diagonal [128, 32] weight; EVERY output column carries a
    valid channel (col c -> channel c%3), so all 128 partitions of the
    staged output hold genuine values (dups don't change min/max) and no
    masking is ever needed.
  - ACT is software-pipelined (tanh2 of t-1 after tanh1 of t) with a shared
    3-buf PSUM pool so tanh runs back-to-back; PE has ~2x slack.
  - global min/max: per-partition running min/max on DVE; 7 partition-fold
    max ops compact [128,2] -> [1,2]; one 8-byte DMA + AllGather + broadcast
    read; normalize split ACT (Identity w/ scale+bias) + DVE in 4-chunk
    pieces with stores interleaved on the Sync/GpSimd DMA queues.
"""

import os
import numpy as np

B, N, NI, H, NO = 4, 262144, 4, 64, 3
NCORES = 8
NPIX_TOT = B * N
NPIX = NPIX_TOT // NCORES      # 131072 pixels per core
NSUB = 16                      # subsets per core
SUBPIX = NPIX // NSUB          # 8192 pixels per subset
CW = 512                       # matmul moving-dim chunk width
NCHUNK = SUBPIX // CW          # 16 chunks per subset
NT = NSUB * NCHUNK // 4        # 64 pair-tiles (4 subsets per tile)
OST_F = 2 * NT // 4 * 512      # 16384 staged cols
F32MAX = 3.0e38

_CACHE = {}
LAST_RESULTS = None            # test.py reads exec_time_ns from here


def _build_module(mm_dtype_name="bfloat16"):
    import concourse.bass as bass
    import concourse.tile as tile
    from concourse import bacc, mybir
    from concourse.tile import add_dep_helper

    from concourse import library_config

    dt = mybir.dt
    alu = mybir.AluOpType
    act = mybir.ActivationFunctionType
    f32 = dt.float32
    mmdt = getattr(dt, mm_dtype_name)

    nc = bacc.Bacc("TRN2", target_bir_lowering=False, debug=False,
                   num_devices=NCORES)

    x_d = nc.dram_tensor("xcore", [128, SUBPIX], mmdt, kind="ExternalInput").ap()
    w1_d = nc.dram_tensor("w1s", [128, 4 * H], mmdt, kind="ExternalInput").ap()
    w2_d = nc.dram_tensor("w2s", [128, H], mmdt, kind="ExternalInput").ap()
    w3_d = nc.dram_tensor("w3bd", [128, 32], mmdt, kind="ExternalInput").ap()
    b1_d = nc.dram_tensor("b1s", [128, 1], f32, kind="ExternalInput").ap()
    b2_d = nc.dram_tensor("b2s", [128, 1], f32, kind="ExternalInput").ap()
    b3_d = nc.dram_tensor("b3s", [128, 1], f32, kind="ExternalInput").ap()
    out_d = nc.dram_tensor("out", [24, OST_F], f32,
                           kind="ExternalOutput").ap()



    with tile.TileContext(nc) as tc:
        with tc.tile_pool(name="const", bufs=1) as const, \
             tc.tile_pool(name="stage", bufs=1) as stage, \
             tc.tile_pool(name="hid", bufs=2) as hid, \
             tc.tile_pool(name="mm", bufs=1) as mmp, \
             tc.tile_pool(name="pmm", bufs=3, space="PSUM") as pmm, \
             tc.tile_pool(name="ps3", bufs=2, space="PSUM") as ps3:

            xin = const.tile([128, SUBPIX], mmdt, tag="xin")
            w1s = const.tile([128, 4 * H], mmdt, tag="w1s")
            w2s = const.tile([128, H], mmdt, tag="w2s")
            w3bd = const.tile([128, 32], mmdt, tag="w3bd")
            b1s = const.tile([128, 1], f32, tag="b1s")
            b2s = const.tile([128, 1], f32, tag="b2s")
            b3s = const.tile([128, 1], f32, tag="b3s")

            nc.sync.dma_start(out=w1s[:], in_=w1_d)
            nc.scalar.dma_start(out=xin[:, 0:CW], in_=x_d[:, 0:CW])
            nc.sync.dma_start(out=b1s[:], in_=b1_d)
            nc.scalar.dma_start(out=w2s[:], in_=w2_d)
            nc.sync.dma_start(out=w3bd[:], in_=w3_d)
            nc.scalar.dma_start(out=b2s[:], in_=b2_d)
            nc.sync.dma_start(out=b3s[:], in_=b3_d)
            nc.scalar.dma_start(out=xin[:, CW:], in_=x_d[:, CW:])

            # staged pre-norm output: partition 32w + 16a + c (channel c%3),
            # free (u//4)*512 + n; u identifies the L3 emission index.
            ostage = stage.tile([128, OST_F], f32, tag="ostage")

            rmin = mmp.tile([128, 1], f32, tag="rmin")
            rmax = mmp.tile([128, 1], f32, tag="rmax")
            nc.vector.memset(rmin[:], F32MAX)
            nc.vector.memset(rmax[:], -F32MAX)
            # constants for the tail (off the critical path): transpose pad,
            # butterfly send buffer pad, ones row for the broadcast matmul
            scr = mmp.tile([128, 32], f32, tag="scr")
            nc.vector.memset(scr[:], -F32MAX)
            bf0 = mmp.tile([128, 1], f32, tag="bf0")
            nc.vector.memset(bf0[:], -F32MAX)
            ones1 = const.tile([1, 128], f32, tag="ones1")
            nc.vector.memset(ones1[:], 1.0)

            # butterfly buffers + sems (used in the tail; descriptors are
            # pre-generated below so the ucode library load, entry barrier,
            # and Q7 desc-gen all overlap the main loop)
            inb = [mmp.tile([128, 1], f32, tag=f"inb{r}", name=f"inb{r}")
                   for r in range(3)]
            bfy = [mmp.tile([128, 1], f32, tag=f"bfy{r}", name=f"bfy{r}")
                   for r in range(3)]
            psem = nc.alloc_semaphore("bf_prep")
            lsem = nc.alloc_semaphore("bf_lsem")
            rsems = [nc.alloc_semaphore(f"bf_rsem{r}") for r in range(3)]
            ssnd = [nc.alloc_semaphore(f"bf_snd{r}") for r in range(3)]
            smm = nc.alloc_semaphore("bf_mm")
            smm2 = nc.alloc_semaphore("bf_mm2")
            curs = [bf0, bfy[0], bfy[1]]

            # pull the remote-dma ucode library load off the tail's critical
            # path (gpsimd is idle during the main loop anyway)
            nc.gpsimd.load_library(library_config.remote_dma)

            def emit_l1(t):
                c = t // 4
                p1 = pmm.tile([128, 2 * CW], f32, tag="pmm",
                              name=f"p1t{t}")
                for v in range(2):
                    for a in range(2):
                        s = 4 * (t % 4) + 2 * v + a
                        g, q = s % 4, s // 4
                        nc.tensor.matmul(
                            out=p1[64 * a: 64 * a + 64, CW * v: CW * v + CW],
                            lhsT=w1s[32 * g: 32 * g + 32, H * q: H * q + H],
                            rhs=xin[32 * g: 32 * g + 32,
                                    c * CW: (c + 1) * CW],
                            start=True, stop=True,
                            tile_position=(32 * g, 64 * a))
                return p1

            def emit_tanh1(t, p1):
                h1 = hid.tile([128, 2 * CW], mmdt, tag="h1")
                nc.scalar.activation(h1[:], p1[:], act.Tanh, bias=b1s[:])
                return h1

            def emit_l2(t, h1):
                p2 = pmm.tile([128, 2 * CW], f32, tag="pmm")
                for v in range(2):
                    for a in range(2):
                        # odd column-half swaps output halves so all four
                        # matmuls pack onto disjoint PE subarray quadrants
                        ao = a ^ (v & 1)
                        nc.tensor.matmul(
                            out=p2[64 * ao: 64 * ao + 64,
                                   CW * v: CW * v + CW],
                            lhsT=w2s[64 * a: 64 * a + 64, :],
                            rhs=h1[64 * a: 64 * a + 64,
                                   CW * v: CW * v + CW],
                            start=True, stop=True,
                            tile_position=(64 * a, 64 * ao))
                return p2

            def emit_tanh2(t, p2):
                h2 = hid.tile([128, 2 * CW], mmdt, tag="h2")
                nc.scalar.activation(h2[:], p2[:], act.Tanh, bias=b2s[:])
                return h2

            ps3_box = [None]

            def emit_l3(t, h2):
                for v in range(2):
                    u = 2 * t + v
                    w = u % 4
                    if w == 0:
                        ps3_box[0] = ps3.tile([128, CW], f32, tag="p3",
                                              name=f"p3t{u}")
                    p3 = ps3_box[0]
                    nc.tensor.matmul(
                        out=p3[32 * w: 32 * w + 32, :],
                        lhsT=w3bd[:],
                        rhs=h2[:, CW * v: CW * v + CW],
                        start=True, stop=True,
                        tile_position=(0, 32 * w))
                    if w == 3:
                        ob = ostage[:, (u // 4) * CW:
                                    (u // 4) * CW + CW]
                        nc.vector.tensor_scalar(ob, p3[:], b3s[:], None,
                                                alu.add)
                        cmin = mmp.tile([128, 1], f32, tag="cmin")
                        cmax = mmp.tile([128, 1], f32, tag="cmax")
                        nc.vector.tensor_reduce(cmin[:], ob,
                                                mybir.AxisListType.X, alu.min)
                        nc.vector.tensor_reduce(cmax[:], ob,
                                                mybir.AxisListType.X, alu.max)
                        nc.vector.tensor_tensor(rmin[:], rmin[:], cmin[:],
                                                alu.min)
                        nc.vector.tensor_tensor(rmax[:], rmax[:], cmax[:],
                                                alu.max)

            # ---- software-pipelined main loop ----
            # PE static order: L1(t+1), L2(t), L3(t-1)  — L1 prefill first
            # ACT static order: tanh1(t), tanh2(t-1)    — back-to-back
            p1s, p2s = {0: emit_l1(0)}, {}
            for t in range(NT + 1):
                if t < NT:
                    h1 = emit_tanh1(t, p1s.pop(t))
                    if t + 1 < NT:
                        p1s[t + 1] = emit_l1(t + 1)
                    p2s[t] = emit_l2(t, h1)
                if t - 1 >= 0:
                    tp = t - 1
                    h2 = emit_tanh2(tp, p2s.pop(tp))
                    emit_l3(tp, h2)

            # ---- global min/max: local compaction + XOR-butterfly ----
            # every partition of ostage holds genuine output values (L3 dup
            # channels), so no masking is needed. Compact the per-partition
            # (-min, max) pairs onto partitions {0,1} via transpose + reduce
            # + cross-block copies, then exchange across the 8 cores with a
            # 3-round recursive-doubling butterfly of SBUF->SBUF remote DMAs
            # (the bir-kernel barrier's prelude AllGather overlaps the main
            # loop, so entry sync costs nothing here).
            nc.vector.tensor_scalar(scr[:, 0:1], rmin[:], -1.0, None,
                                    alu.mult)
            nc.vector.tensor_copy(scr[:, 1:2], rmax[:])
            ttr = mmp.tile([128, 32], f32, tag="ttr")
            nc.vector.transpose(ttr[:], scr[:])
            red = mmp.tile([128, 1], f32, tag="red")
            nc.vector.tensor_reduce(red[:], ttr[:], mybir.AxisListType.X,
                                    alu.max)
            red2 = mmp.tile([32, 4], f32, tag="red2")
            for b in range(4):
                nc.vector.tensor_copy(red2[0:32, b: b + 1],
                                      red[32 * b: 32 * b + 32, 0:1])
            nc.vector.tensor_reduce(bf0[0:2, :], red2[0:2, :],
                                    mybir.AxisListType.X, alu.max)

            ttr2 = mmp.tile([128, 32], f32, tag="ttr2")
            rng = mmp.tile([1, 1], f32, tag="rng")
            iv = mmp.tile([1, 2], f32, tag="iv")
            ivs = mmp.tile([128, 2], f32, tag="ivs")
            ivb = ps3.tile([128, CW], f32, tag="p3")

            with tc.tile_critical(name="bfly"):
                # gpsimd: generate all round descriptors, then fire each
                # round's send as soon as its source is combined
                nc.gpsimd.bir_kernel_barrier_wait([list(range(NCORES))])
                for r in range(3):
                    delta = 1 << r
                    rd = [None] * 8
                    rd[delta] = (0, delta)
                    nc.gpsimd.remote_dma_broadcast(
                        out_ap=inb[r][:], in_ap=curs[r][:],
                        remote_sem=rsems[r], local_sem=lsem,
                        rdests=rd).then_inc(psem, 1)
                nc.gpsimd.wait_ge(psem, 3)
                for r in range(3):
                    nc.gpsimd.wait_ge(ssnd[r], 1)
                    nc.gpsimd.trigger_dma(count=1)
                # DVE: combine rounds as partner data lands; a chain sem
                # orders same-engine RAW edges (critical sections get no
                # automatic syncs)
                sdve = nc.alloc_semaphore("bf_dve")
                cnt = [0]

                def dve(inst):
                    if cnt[0] > 0:
                        inst._wait_ge(sdve, cnt[0])
                    inst.then_inc(sdve, 1)
                    cnt[0] += 1
                    return inst

                nc.vector.sem_inc(ssnd[0], 1)
                cur = bf0
                for r in range(3):
                    nc.vector.wait_ge(rsems[r], 2)
                    dve(nc.vector.tensor_tensor(bfy[r][:], cur[:],
                                                inb[r][:], alu.max))
                    if r < 2:
                        nc.vector.sem_inc(ssnd[r + 1], 1)._wait_ge(
                            sdve, cnt[0])
                    cur = bfy[r]
                # partition 0 = -gmin, partition 1 = gmax -> transpose col 0
                # so both land on partition 0's free dim, compute (inv, off),
                # broadcast to all partitions via a rank-1 ones matmul
                dve(nc.vector.tensor_copy(scr[:, 0:1], cur[:]))
                dve(nc.vector.transpose(ttr2[:], scr[:]))
                dve(nc.vector.tensor_tensor(rng[:], ttr2[0:1, 0:1],
                                            ttr2[0:1, 1:2], alu.add))
                dve(nc.vector.reciprocal(iv[0:1, 0:1], rng[:]))
                dve(nc.vector.tensor_tensor(
                    iv[0:1, 1:2], ttr2[0:1, 0:1], iv[0:1, 0:1],
                    alu.mult))
                nc.vector.sem_inc(smm, 1)._wait_ge(sdve, cnt[0])
                nc.tensor.wait_ge(smm, 1)
                nc.tensor.matmul(
                    out=ivb[0:128, 0:2], lhsT=ones1[0:1, 0:128],
                    rhs=iv[0:1, 0:2], start=True,
                    stop=True).then_inc(smm2, 1)
                nc.vector.wait_ge(smm2, 1)
                dve(nc.vector.tensor_copy(ivs[:], ivb[0:128, 0:2]))
            inv = ivs[:, 0:1]
            off = ivs[:, 1:2]

            # ---- normalize + store (min/max make clip a no-op up to 1-ulp
            # rounding, matching the reference's clip) ----
            # 8 column chunks: ACT normalizes chunks 0-3, DVE chunks 4-7, in
            # parallel; each chunk's store DMA fires as soon as it's done.
            # ostage valid rows for DRAM: 32w+16a+4o (o = channel).
            NCH = 4
            CHW = OST_F // NCH
            for ch in range(NCH):
                fs = CHW * ch
                oc = ostage[:, fs: fs + CHW]
                nc.vector.tensor_scalar(oc, oc, inv, off,
                                        alu.mult, alu.add)
                eng = nc.sync if ch % 2 == 0 else nc.gpsimd
                for m in range(8):
                    p0 = 16 * m
                    sl = ostage[p0: p0 + 12, fs: fs + CHW]
                    sl = sl.rearrange("(o r) f -> o r f", o=3)[:, 0, :]
                    eng.dma_start(
                        out=out_d[3 * m: 3 * m + 3, fs: fs + CHW],
                        in_=sl)
    nc.compile()
    return nc


def _host_inputs(x, W1, b1, W2, b2, W3, b3, mm_np=None):
    """Repack full inputs into per-core in_maps (host-side, not HW-timed)."""
    if mm_np is None:
        import ml_dtypes
        mm = os.environ.get("CPPN_MM_DTYPE", "bfloat16")
        mm_np = ml_dtypes.bfloat16 if mm == "bfloat16" else np.float32
    x = np.asarray(x, np.float32).reshape(NPIX_TOT, NI)
    W1 = np.asarray(W1, np.float32)
    b1 = np.asarray(b1, np.float32)
    W2 = np.asarray(W2, np.float32)
    b2 = np.asarray(b2, np.float32)
    W3 = np.asarray(W3, np.float32)
    b3 = np.asarray(b3, np.float32)

    blk = np.zeros((32, 4 * H), np.float32)
    for q in range(4):
        blk[8 * q: 8 * q + 4, H * q: H * q + H] = W1
        blk[8 * q + 4: 8 * q + 8, H * q: H * q + H] = W1
    w1s = np.tile(blk, (4, 1))

    w2s = np.concatenate([W2, W2], axis=0)

    # L3: every column valid — col c (within each 16-col half) = channel c%3
    w3bd = np.zeros((128, 32), np.float32)
    for c in range(16):
        w3bd[0:64, c] = W3[:, c % 3]
        w3bd[64:128, 16 + c] = W3[:, c % 3]

    b1s = np.concatenate([b1, b1])[:, None].astype(np.float32)
    b2s = np.concatenate([b2, b2])[:, None].astype(np.float32)
    b3s = np.empty((128, 1), np.float32)
    for p in range(128):
        b3s[p, 0] = b3[(p % 16) % 3]

    in_maps = []
    x_hi = x.astype(mm_np)
    x_lo = (x - x_hi.astype(np.float32)).astype(mm_np)
    for k in range(NCORES):
        sh_hi = x_hi[k * NPIX: (k + 1) * NPIX].reshape(NSUB, SUBPIX, NI)
        sh_lo = x_lo[k * NPIX: (k + 1) * NPIX].reshape(NSUB, SUBPIX, NI)
        xcore = np.empty((128, SUBPIX), mm_np)
        for s in range(NSUB):
            g, q = s % 4, s // 4
            p0 = 32 * g + 8 * q
            xcore[p0: p0 + 4, :] = sh_hi[s].T
            xcore[p0 + 4: p0 + 8, :] = sh_lo[s].T
        in_maps.append({
            "xcore": np.ascontiguousarray(xcore),
            "w1s": w1s.astype(mm_np), "w2s": w2s.astype(mm_np),
            "w3bd": w3bd.astype(mm_np),
            "b1s": b1s, "b2s": b2s, "b3s": b3s,
        })
    return in_maps


def _unshard(core_outs):
    """[24, OST_F] per core -> [NO, B, N] full output.

    Row j = 6w + 3a + o; col = (u//4)*512 + n with u = 4*blk + w the L3
    emission index; u = 2t + v; subset s = 4*(t%4) + 2v + (a^v) (the
    layer-2 diagonal packing swaps halves on odd column-halves), chunk
    c = t//4; pixel = s*SUBPIX + c*512 + n.
    """
    out = np.empty((NO, NPIX_TOT), np.float32)
    for k in range(NCORES):
        arr = np.asarray(core_outs[k]).reshape(24, OST_F // 512, 512)
        for j in range(24):
            w, a, o = j // 6, (j % 6) // 3, j % 3
            for blk in range(OST_F // 512):
                u = 4 * blk + w
                t, v = u // 2, u % 2
                s = 4 * (t % 4) + 2 * v + (a ^ v)
                c = t // 4
                base = k * NPIX + s * SUBPIX + c * 512
                out[o, base: base + 512] = arr[j, blk, :]
    return out.reshape(NO, B, N)


def kernel(x, W1, b1, W2, b2, W3, b3):
    global LAST_RESULTS
    from concourse.bass_utils import run_bass_kernel_spmd

    mm = os.environ.get("CPPN_MM_DTYPE", "bfloat16")
    if mm not in _CACHE:
        _CACHE[mm] = _build_module(mm)
    nc = _CACHE[mm]

    in_maps = _host_inputs(x, W1, b1, W2, b2, W3, b3)
    res = run_bass_kernel_spmd(nc, in_maps, list(range(NCORES)))
    LAST_RESULTS = res
    return _unshard([res.results[k]["out"] for k in range(NCORES)])
